# revision 20
# baseline (speedup 1.0000x reference)
"""Trainium2 Bass kernel for nn_BiLinearInteractionLayer (fp16 fast path).

Math: x:(B=4096, F=32, D=64) f32, W:(P=496, D=64, D=64) f32 (torch Linear
layout).  For each pair p=(i,j), i<j:
    out[b, p, e] = (sum_d x[b,i,d] * W[p,e,d]) * x[b,j,e]

The harness gate is rel_err < 2e-2 (max-abs / max-scale).  The original
kernel computed to 2.7e-7 with an exact hi/lo fp16 expansion and stored
fp32 output -- but it is HBM-bound (65 of 77 MB/core is the output
store).  This version computes in fp16 (~8e-4 rel err, ~25x inside the
gate) and halves the dominant traffic:

  per core: x fp16 2MB + xT fp16 2MB + W^T fp16 3.9MB + out fp16 32.5MB
  = 40.4MB vs 77MB before, at the ~358 GB/s HBM-per-core limit.

Design (data-parallel over batch, 8 cores x 512 rows):

* Host precomputes fp16 x in natural layout, fp16 x^T per batch tile in
  a per-field-pair layout (field 2g on rows 0:63, field 2g+1 on rows
  64:127), and fp16 W^T -- no on-chip transposes.
* Matmuls are k=128: the stationary is one PAIR of fields' x^T, and the
  streamed weight tile has the other field's 64 rows zeroed.  k=128
  keeps the PE HAM monitor un-throttled at 2.4 GHz (k=64 under-reports
  and pins 1.2 GHz -- measured +35us), and LDWEIGHTS overlaps matmuls.
  Row-group packing (tile_position) was rejected: two concurrent
  row-group matmuls draining into one PSUM bank are a fatal HW error
  (verified by bisection) and the output layout can't keep them apart.
* Weights live in 4 SBUF tiles (even/odd fields x groups 0-3 / 4-15,
  data on partitions 0:63 / 64:127 resp.), each zero-half initialized
  by ONE big DVE memset (~3us) and filled by ONE DMA.  An earlier
  variant used 31 per-group memsets + 31 DMAs: the DVE DRAIN per memset
  plus the sync ring's ~620ns per-DMA issue cost ~35us of startup
  serialization.  Splitting even/odd at group 4 gives window 0 its
  weights after ~2 DMAs while the rest stream in behind.
* TRN2 matmul can only write fp32 PSUM.  Evacuation + elementwise
  multiply, per 2048-col window (4 PSUM banks):
    path B (most windows): ACT copies PSUM->SBUF fp16 ((172+FD)/1.2GHz),
      then DVE tensor_mul fp16*fp16->fp16 at 2x_1P.
    path A (PATH_A windows): DVE multiplies straight from fp32 PSUM at
      1x (mixed-dtype tensor_tensor with an fp16 SBUF operand works).
  The split keeps ACT and DVE balanced under the DMA floor.
* All loads AND stores go on the Sync HWDGE ring; stores must not be
  issued from ACT (a store waiting on DVE in the ACT FIFO queue blocks
  the next window's PSUM copy -- cost ~25us in an earlier variant).
  bt+1's loads are issued before bt's stores so the FIFO ring never
  parks a ready load behind a store that waits on compute.
"""
import numpy as np
from bisect import bisect_right

import concourse.bacc as bacc
import concourse.tile as tile
import concourse.mybir as mybir
from concourse.bass_utils import run_bass_kernel_spmd

B = 4096
F = 32
D = 64
P = F * (F - 1) // 2  # 496
N_CORES = 8
BL = B // N_CORES     # 512 rows per core
BT = 128              # batch tile (SBUF partitions)
NBT = BL // BT        # 4 batch tiles per core
NCOL = P * D          # 31744 output columns per row
NGRP = F // 2         # 16 field-pair groups
GSPLIT = 4            # weight tiles split at this group
WIN = 2048            # evacuation window = 4 fp32 PSUM banks
NWIN = (NCOL + WIN - 1) // WIN  # 16 (last window 1024)
PATH_A = (4, 9, 14)   # windows multiplied straight from PSUM (DVE 1x)

f32 = mybir.dt.float32
f16 = mybir.dt.float16

_nc_cache = None


def _off(i):
    """Number of pairs with left field < i."""
    return 31 * i - i * (i - 1) // 2


_FLD_START = [_off(i) * D for i in range(F)]            # canonical col starts
_FLD_W = [(31 - i) * D for i in range(F)]               # field block widths
_GRP_START = [_FLD_START[2 * g] for g in range(NGRP)]   # group col starts
_MM_BOUNDS = sorted(set(range(0, NCOL, 512)) | set(_GRP_START))
_FLD_BOUNDS = _FLD_START[1:]


def _segments(w0, w1, bounds):
    pts = [w0] + [b for b in bounds if w0 < b < w1] + [w1]
    return list(zip(pts[:-1], pts[1:]))


def _build():
    nc = bacc.Bacc("TRN2", target_bir_lowering=False, debug=False,
                   num_devices=N_CORES)
    x_in = nc.dram_tensor("x16", [BL, F * D], f16, kind="ExternalInput").ap()
    xt_in = nc.dram_tensor("xt", [128, NBT * NGRP * BT], f16,
                           kind="ExternalInput").ap()
    # zero-padded canonical layout: even-field cols carry data on rows
    # 0:63 (zeros below), odd-field cols on rows 64:127 (zeros above)
    wt_in = nc.dram_tensor("wt", [128, NCOL], f16, kind="ExternalInput").ap()
    out = nc.dram_tensor("out", [BL, NCOL], f16, kind="ExternalOutput").ap()

    with tile.TileContext(nc) as tc:
        with (
            tc.tile_pool(name="consts", bufs=1) as consts,
            tc.tile_pool(name="xp", bufs=2) as xp,
            tc.tile_pool(name="xtp", bufs=2) as xtp,
            tc.tile_pool(name="mmp", bufs=3) as mmp,
            tc.tile_pool(name="otp", bufs=3) as otp,
            tc.tile_pool(name="psm", bufs=2, space="PSUM") as psm,
        ):
            # one weight tile per field-pair group, canonical col order,
            # zeros shipped from DRAM (the early DMA timeline is starved
            # -- the +4MB of zeros rides free there, and it removes the
            # memset -> W-DMA -> matmul startup chain that cost ~12us)
            wt_g = []
            for g in range(NGRP):
                t = consts.tile([128, _FLD_W[2 * g] + _FLD_W[2 * g + 1]],
                                f16, tag=f"wt{g}")
                wt_g.append(t)

            def load_bt(bt):
                x16 = xp.tile([BT, F * D], f16, tag="x")
                nc.sync.dma_start(out=x16,
                                  in_=x_in[bt * BT:(bt + 1) * BT, :])
                c0 = bt * NGRP * BT
                xT = xtp.tile([128, NGRP * BT], f16, tag="xT")
                nc.sync.dma_start(out=xT, in_=xt_in[:, c0:c0 + NGRP * BT])
                return x16, xT

            tiles = load_bt(0)
            for g in range(NGRP):
                c0 = _GRP_START[g]
                c1 = c0 + _FLD_W[2 * g] + _FLD_W[2 * g + 1]
                nc.sync.dma_start(out=wt_g[g], in_=wt_in[:, c0:c1])

            for bt in range(NBT):
                r0, r1 = bt * BT, (bt + 1) * BT
                x16, xT = tiles
                if bt + 1 < NBT:
                    tiles = load_bt(bt + 1)

                # stores span 4 windows (2MB) for SWDGE efficiency; the
                # last batch tile stores every 2 windows to shorten the
                # final drain after compute ends
                span = 2 if bt == NBT - 1 else 4
                ot = None
                o0 = 0
                for w in range(NWIN):
                    w0 = w * WIN
                    w1 = min(w0 + WIN, NCOL)
                    wl = w1 - w0
                    if w % span == 0:
                        o0 = w0
                        ot = otp.tile([BT, 4 * WIN], f16, tag="ot")

                    pm = psm.tile([BT, WIN], f32, tag="mm")
                    for (s0, s1) in _segments(w0, w1, _MM_BOUNDS):
                        g = bisect_right(_GRP_START, s0) - 1
                        nc.tensor.matmul(
                            pm[:, s0 - w0:s1 - w0],
                            xT[:, g * BT:(g + 1) * BT],
                            wt_g[g][:, s0 - _GRP_START[g]:
                                    s1 - _GRP_START[g]],
                            start=True, stop=True)

                    if w in PATH_A:
                        src = pm
                    else:
                        src = mmp.tile([BT, WIN], f16, tag="m16")
                        nc.scalar.copy(src[:, :wl], pm[:, :wl])
                    for (s0, s1) in _segments(w0, w1, _FLD_BOUNDS):
                        i = bisect_right(_FLD_START, s0) - 1
                        xc = (i + 1) * D + (s0 - _FLD_START[i])
                        nc.vector.tensor_mul(
                            ot[:, s0 - o0:s1 - o0],
                            src[:, s0 - w0:s1 - w0],
                            x16[:, xc:xc + (s1 - s0)])

                    if w % span == span - 1 or w == NWIN - 1:
                        ol = w1 - o0
                        nc.gpsimd.dma_start(out=out[r0:r1, o0:o0 + ol],
                                            in_=ot[:, :ol])
    nc.compile()
    return nc


def _get_nc():
    global _nc_cache
    if _nc_cache is None:
        _nc_cache = _build()
    return _nc_cache


def _prep_inputs(x, W):
    x16 = np.asarray(x, dtype=np.float16)            # (B, F, D)
    xs = np.ascontiguousarray(x16.reshape(N_CORES, BL, F * D))
    # xt[c, h*64+d, bt*2048 + g*128 + b] = x[c, bt*128+b, 2g+h, d]
    xr = x16.reshape(N_CORES, NBT, BT, NGRP, 2, D)
    xt = np.ascontiguousarray(xr.transpose(0, 4, 5, 1, 3, 2)).reshape(
        N_CORES, 128, NBT * NGRP * BT)
    # canonical W^T: wtc[d, p*64+e] = W[p, e, d]
    wtc = np.ascontiguousarray(
        np.asarray(W, dtype=np.float32).transpose(2, 0, 1).reshape(D, NCOL)
    ).astype(np.float16)
    # zero-padded: even-field cols on rows 0:63, odd-field on rows 64:127
    wt2 = np.zeros((128, NCOL), dtype=np.float16)
    for f in range(F - 1):
        r0 = 0 if f % 2 == 0 else D
        wt2[r0:r0 + D, _FLD_START[f]:_FLD_START[f] + _FLD_W[f]] = \
            wtc[:, _FLD_START[f]:_FLD_START[f] + _FLD_W[f]]
    return xs, xt, wt2


def _run(x, W, trace=False, trace_kwargs=None):
    xs, xt, wt2 = _prep_inputs(x, W)
    in_maps = [{"x16": xs[c], "xt": xt[c], "wt": wt2} for c in range(N_CORES)]
    res = run_bass_kernel_spmd(_get_nc(), in_maps, list(range(N_CORES)),
                               trace=trace, **(trace_kwargs or {}))
    outs = [res.results[c]["out"].astype(np.float32).reshape(BL, P, D)
            for c in range(N_CORES)]
    return np.concatenate(outs, axis=0), res


def kernel(x, W):
    out, _ = _run(x, W)
    return out
